# revision 29
# baseline (speedup 1.0000x reference)
"""Gemma3 sliding-window attention on 8 trn2 NeuronCores (Bass/Tile).

Sharding: core c -> batch b=c//4, kv-head j=c%4 (query heads 2j,2j+1).
Each core computes d-major qT/kT projections (no transposes needed for
scores), token-major v, flash-style sT=[tk,tq] attention blocks over the
1024-token sliding window, and a partial output projection outT[2560,2048].
Host sums the 4 partials per batch and transposes back.
"""

import os
import sys

for _p in ("/opt/trn_rl_repo", "/root/.axon_site/_ro/trn_rl_repo"):
    if _p not in sys.path:
        sys.path.append(_p)

import numpy as np
import ml_dtypes

import concourse.bass as bass
import concourse.bacc as bacc
import concourse.mybir as mybir
from concourse.tile import TileContext
from concourse.bass_utils import run_bass_kernel_spmd

B, T, H = 2, 2048, 2560
NH, NKV, D = 8, 4, 256
WINDOW = 1024
EPS = 1e-6
ROPE_THETA = 10000.0
SCALE = D ** -0.5

F32 = mybir.dt.float32
BF16 = mybir.dt.bfloat16

NCORES = 8
HK = 20          # H / 128 contraction tiles
TQ = 512         # query tile (free dim)
NTQ = T // TQ    # 4
TK = 128         # key tile (partition dim)
DQ = 512         # q cols per core (2 heads)
DK = 256         # k/v cols per core (1 kv head)

AFT = mybir.ActivationFunctionType

bf16 = lambda a: np.ascontiguousarray(a).astype(ml_dtypes.bfloat16)


def _attn_plan(mask2d):
    """Static block plan from the [T,T] bool mask.

    Returns (patterns [n,TK,128] float32 0/1 (zero-padded), plan[tt] = list of
    (k, lo, hi, mid, mlo, mhi)): block covers valid tq columns [lo,hi) of the
    tile; mask pattern mid multiplies local columns [mlo,mhi) (width <= 128).
    Blocks in sT layout [tk, tq]."""
    pat_index = {}
    patterns = []
    plan = []
    for tt in range(NTQ):
        q0 = tt * TQ
        blocks = []
        for k in range(T // TK):
            k0 = k * TK
            blk = mask2d[q0:q0 + TQ, k0:k0 + TK]   # [TQ, TK]
            cols_any = blk.any(axis=1)
            if not cols_any.any():
                continue
            idx = np.nonzero(cols_any)[0]
            lo, hi = int(idx[0]), int(idx[-1]) + 1
            assert cols_any[lo:hi].all(), "non-contiguous valid column range"
            cols_all = blk.all(axis=1)
            partial = cols_any & ~cols_all
            if partial.any():
                pidx = np.nonzero(partial)[0]
                mlo, mhi = int(pidx[0]), int(pidx[-1]) + 1
                assert mhi - mlo <= 128, "partial range wider than 128"
                sub = blk.T[:, mlo:mhi].astype(np.float32)   # [TK, w]
                padded = np.zeros((TK, 128), np.float32)
                padded[:, :sub.shape[1]] = sub
                key = padded.tobytes()
                if key not in pat_index:
                    pat_index[key] = len(patterns)
                    patterns.append(padded)
                blocks.append((k, lo, hi, pat_index[key], mlo, mhi))
            else:
                blocks.append((k, lo, hi, -1, 0, 0))
        plan.append(blocks)
    if not patterns:
        patterns = [np.ones((TK, 128), np.float32)]
    return np.stack(patterns), plan


def build_nc(nmask, plan, apply_qs, apply_ks):
    nc = bacc.Bacc("TRN2", target_bir_lowering=False, debug=False)

    xT = nc.dram_tensor("xT", [NTQ, 128, HK, TQ], BF16, kind="ExternalInput")
    wq = nc.dram_tensor("wq", [128, HK, DQ], BF16, kind="ExternalInput")
    wk = nc.dram_tensor("wk", [128, HK, DK], BF16, kind="ExternalInput")
    wv = nc.dram_tensor("wv", [128, HK, DK], BF16, kind="ExternalInput")
    wo = nc.dram_tensor("wo", [128, 4, H], BF16, kind="ExternalInput")
    cosd = nc.dram_tensor("cosd", [D // 2, T], BF16, kind="ExternalInput")
    sind = nc.dram_tensor("sind", [D // 2, T], BF16, kind="ExternalInput")
    maskt = nc.dram_tensor("maskt", [128, nmask, 128], BF16, kind="ExternalInput")
    qsc = nc.dram_tensor("qsc", [D, 1], F32, kind="ExternalInput")
    ksc = nc.dram_tensor("ksc", [D, 1], F32, kind="ExternalInput")
    outT = nc.dram_tensor("outT", [H, T], BF16, kind="ExternalOutput")

    with TileContext(nc) as tc:
        nc_lp = nc.allow_low_precision(reason="bf16 throughout; tol 2e-2")
        nc_lp.__enter__()
        with (
            tc.tile_pool(name="const", bufs=1) as constp,
            tc.tile_pool(name="wts", bufs=1) as wtsp,
            tc.tile_pool(name="xt", bufs=2) as xtp,
            tc.tile_pool(name="qkv", bufs=1) as qkvp,
            tc.tile_pool(name="w1", bufs=1) as w1,
            tc.tile_pool(name="w2", bufs=2) as w2,
            tc.tile_pool(name="w3", bufs=3) as w3,
            tc.tile_pool(name="ostp", bufs=6) as ostp,
            tc.tile_pool(name="attn", bufs=2) as attnp,
        ):
            from contextlib import contextmanager

            @contextmanager
            def deferred(off=200000):
                tc.cur_priority += off
                try:
                    yield
                finally:
                    tc.cur_priority -= off

            # ---- static loads (ordered by first use) ----
            wq_sb = wtsp.tile([128, HK, DQ], BF16, tag="wq")
            for c0, c1 in ((0, 2), (2, 7), (7, 13), (13, 20)):
                nc.sync.dma_start(wq_sb[:, c0:c1, :], wq[:, c0:c1, :])
            wk_sb = wtsp.tile([128, HK, DK], BF16, tag="wk")
            wv_sb = wtsp.tile([128, HK, DK], BF16, tag="wv")
            cos_sb = constp.tile([128, T], BF16, tag="cos")
            sin_sb = constp.tile([128, T], BF16, tag="sin")
            qsc_sb = constp.tile([128, 2, 1], F32, tag="qsc")
            ksc_sb = constp.tile([128, 2, 1], F32, tag="ksc")
            with deferred(60):
                nc.sync.dma_start(wk_sb[:], wk[:])
                nc.sync.dma_start(wv_sb[:], wv[:])
                nc.sync.dma_start(cos_sb[:], cosd[:])
                nc.sync.dma_start(sin_sb[:], sind[:])
                nc.sync.dma_start(qsc_sb[:],
                                  qsc.rearrange("(n p) o -> p n o", p=128))
                nc.sync.dma_start(ksc_sb[:],
                                  ksc.rearrange("(n p) o -> p n o", p=128))
            mask_sb = constp.tile([128, nmask, 128], BF16, tag="mask")
            wo_sb = wtsp.tile([128, 4, H], BF16, tag="wo")
            with deferred(2000):
                nc.sync.dma_start(mask_sb[:], maskt[:])
                nc.sync.dma_start(wo_sb[:], wo[:])

            ones_col = constp.tile([128, 1], F32, tag="ones_col")
            nc.vector.memset(ones_col[:], 1.0)
            ones_col_bf = constp.tile([128, 1], BF16, tag="ones_col_bf")
            nc.vector.memset(ones_col_bf[:], 1.0)
            ones_row = constp.tile([1, 128], F32, tag="ones_row")
            nc.vector.memset(ones_row[:], 1.0)
            eps_sb = constp.tile([1, 1], F32, tag="eps")
            nc.vector.memset(eps_sb[:], EPS)
            eps_sb128 = constp.tile([128, 1], F32, tag="eps128")
            nc.vector.memset(eps_sb128[:], EPS)
            epsD_sb = constp.tile([128, 1], F32, tag="epsD")
            nc.vector.memset(epsD_sb[:], float(D) * EPS)

            # PE warmup while startup DMAs stream (HAM un-throttle)
            with tc.tile_pool(name="warm", bufs=1) as warmp, \
                 tc.tile_pool(name="warmps", bufs=1, space="PSUM") as warmpsp:
                wsrc = warmp.tile([128, 512], BF16, tag="wsrc")
                nc.vector.memset(wsrc[:], 0.0)
                wps = warmpsp.tile([128, 512], F32, tag="wps")
                for wi in range(42):
                    nc.tensor.matmul(wps[:], wsrc[:, 0:128], wsrc[:],
                                     start=(wi == 0), stop=(wi == 41))

            # persistent per-core tensors
            qTn = qkvp.tile([128, 4, T], BF16, tag="qTn")    # d-major, 4 d-tiles
            kTn = qkvp.tile([128, 2, T], BF16, tag="kTn")    # d-major, 2 d-tiles
            v_sb = qkvp.tile([128, T // TK, DK], BF16, tag="v")  # token-major

            # ---- phase 1: projections + rmsnorm + rope ----
            def rope_q(ps_d0, ps_d1, cos_t, sin_t, out, o0, o1, sc, apply_sc):
                """q path: evacuate PSUM, rmsnorm (PE colsum + gpsimd bcast),
                rope, normalize, write bf16 d-major."""
                q0 = w2.tile([128, TQ], BF16, tag="qp0")
                nc.scalar.copy(q0[:], ps_d0[:])
                q1 = w2.tile([128, TQ], BF16, tag="qp1")
                nc.vector.tensor_copy(q1[:], ps_d1[:])
                sq0 = w2.tile([128, TQ], BF16, tag="sq0")
                nc.vector.tensor_mul(sq0[:], q0[:], q0[:])
                sq1 = w2.tile([128, TQ], BF16, tag="sq1")
                nc.vector.tensor_mul(sq1[:], q1[:], q1[:])
                ssq = psS.tile([1, TQ], F32, tag="ssq")
                with deferred(500):
                    nc.tensor.matmul(ssq[:], ones_col_bf[:], sq0[:],
                                     start=True, stop=False)
                    nc.tensor.matmul(ssq[:], ones_col_bf[:], sq1[:],
                                     start=False, stop=True)
                rnorm = w1.tile([1, TQ], BF16, tag="rnorm")
                nc.scalar.activation(rnorm[:], ssq[:], AFT.Abs_reciprocal_sqrt,
                                     bias=eps_sb[:], scale=1.0 / D)
                bc = w2.tile([128, TQ], BF16, tag="bcs")
                nc.gpsimd.partition_broadcast(bc[:], rnorm[:])
                # rope halves: h0' = h0*cos - h1*sin ; h1' = h1*cos + h0*sin
                t0 = w2.tile([128, TQ], BF16, tag="t0")
                nc.vector.tensor_mul(t0[:], q0[:], cos_t)
                t1 = w2.tile([128, TQ], BF16, tag="t1")
                nc.vector.tensor_mul(t1[:], q1[:], sin_t)
                nc.vector.tensor_sub(t0[:], t0[:], t1[:])
                if apply_sc:
                    nc.vector.tensor_scalar_mul(t0[:], t0[:], sc[:, 0, :])
                nc.vector.tensor_mul(out[:, o0, :], t0[:], bc[:])
                t2 = w2.tile([128, TQ], BF16, tag="t2")
                nc.vector.tensor_mul(t2[:], q1[:], cos_t)
                t3 = w2.tile([128, TQ], BF16, tag="t3")
                nc.vector.tensor_mul(t3[:], q0[:], sin_t)
                nc.vector.tensor_add(t2[:], t2[:], t3[:])
                if apply_sc:
                    nc.vector.tensor_scalar_mul(t2[:], t2[:], sc[:, 1, :])
                nc.vector.tensor_mul(out[:, o1, :], t2[:], bc[:])

            def rope_k(ps_d0, ps_d1, cos_t, sin_t, out, sc, apply_sc, tt):
                """k path: rope only; k-norm is folded into the exp scale
                later. Computes per-key 1/rms columns into rnk_all."""
                q0 = w2.tile([128, TQ], BF16, tag="qp0")
                nc.scalar.copy(q0[:], ps_d0[:])
                q1 = w2.tile([128, TQ], BF16, tag="qp1")
                nc.vector.tensor_copy(q1[:], ps_d1[:])
                t0 = w2.tile([128, TQ], BF16, tag="t0")
                nc.vector.tensor_mul(t0[:], q0[:], cos_t)
                t1 = w2.tile([128, TQ], BF16, tag="t1")
                nc.vector.tensor_mul(t1[:], q1[:], sin_t)
                if apply_sc:
                    nc.vector.tensor_sub(t0[:], t0[:], t1[:])
                    nc.vector.tensor_scalar_mul(out[:, 0, :], t0[:], sc[:, 0, :])
                else:
                    nc.vector.tensor_sub(out[:, 0, :], t0[:], t1[:])
                t2 = w2.tile([128, TQ], BF16, tag="t2")
                nc.vector.tensor_mul(t2[:], q1[:], cos_t)
                t3 = w2.tile([128, TQ], BF16, tag="t3")
                nc.vector.tensor_mul(t3[:], q0[:], sin_t)
                if apply_sc:
                    nc.vector.tensor_add(t2[:], t2[:], t3[:])
                    nc.vector.tensor_scalar_mul(out[:, 1, :], t2[:], sc[:, 1, :])
                else:
                    nc.vector.tensor_add(out[:, 1, :], t2[:], t3[:])
                # per-key 1/rms columns (post-rope k; rope preserves norms
                # exactly, and scale multiplies elementwise after norm)
                sk0 = w2.tile([128, TQ], BF16, tag="sk0")
                nc.vector.tensor_mul(sk0[:], out[:, 0, :], out[:, 0, :])
                sk1 = w2.tile([128, TQ], BF16, tag="sk1")
                nc.vector.tensor_mul(sk1[:], out[:, 1, :], out[:, 1, :])
                for st in range(TQ // TK):
                    ssqk = psK.tile([128, 1], F32, tag="ssqk")
                    sl = slice(st * TK, (st + 1) * TK)
                    with deferred(300):
                        nc.tensor.matmul(ssqk[:], sk0[:, sl], ones_col_bf[:],
                                         start=True, stop=False)
                        nc.tensor.matmul(ssqk[:], sk1[:, sl], ones_col_bf[:],
                                         start=False, stop=True)
                    nc.scalar.activation(rnk_all[:, tt * 4 + st], ssqk[:],
                                         AFT.Abs_reciprocal_sqrt,
                                         bias=epsD_sb[:], scale=1.0)

            rnk_all = constp.tile([128, T // TK, 1], F32, tag="rnk")
            with (
                tc.tile_pool(name="psP", bufs=3, space="PSUM") as psP,
                tc.tile_pool(name="psV", bufs=2, space="PSUM") as psV,
                tc.tile_pool(name="psS1", bufs=1, space="PSUM") as psS,
                tc.tile_pool(name="psK", bufs=2, space="PSUM") as psK,
            ):
                for tt in range(NTQ):
                    ts = slice(tt * TQ, (tt + 1) * TQ)
                    xt_t = xtp.tile([128, HK, TQ], BF16, tag="xt")
                    xt_chunks = ((0, 3), (3, 8), (8, 14), (14, 20)) if tt == 0 \
                        else ((0, 10), (10, 20))
                    for c0, c1 in xt_chunks:
                        nc.sync.dma_start(xt_t[:, c0:c1, :], xT[tt, :, c0:c1, :])
                    cos_t = cos_sb[:, ts]
                    sin_t = sin_sb[:, ts]
                    # q heads: d-tiles (2h, 2h+1) per head h
                    for h in range(2):
                        pd = []
                        for dh in range(2):
                            dt = 2 * h + dh
                            ps = psP.tile([128, TQ], F32, tag="projq")
                            for ki in range(HK):
                                nc.tensor.matmul(
                                    ps[:],
                                    wq_sb[:, ki, dt * 128:(dt + 1) * 128],
                                    xt_t[:, ki, :],
                                    start=(ki == 0), stop=(ki == HK - 1))
                            pd.append(ps)
                        rope_q(pd[0], pd[1], cos_t, sin_t,
                               qTn[:, :, ts], 2 * h, 2 * h + 1,
                               qsc_sb, apply_qs)
                    # k
                    pd = []
                    for dh in range(2):
                        ps = psP.tile([128, TQ], F32, tag="projq")
                        for ki in range(HK):
                            nc.tensor.matmul(
                                ps[:], wk_sb[:, ki, dh * 128:(dh + 1) * 128],
                                xt_t[:, ki, :],
                                start=(ki == 0), stop=(ki == HK - 1))
                        pd.append(ps)
                    rope_k(pd[0], pd[1], cos_t, sin_t,
                           kTn[:, :, ts], ksc_sb, apply_ks, tt)
                    # v (token-major)
                    for st in range(TQ // TK):
                        ps = psV.tile([128, DK], F32, tag="projv")
                        for ki in range(HK):
                            nc.tensor.matmul(
                                ps[:], xt_t[:, ki, st * 128:(st + 1) * 128],
                                wv_sb[:, ki, :],
                                start=(ki == 0), stop=(ki == HK - 1))
                        nc.vector.tensor_copy(v_sb[:, tt * 4 + st, :], ps[:])

            # ---- phase 2: attention + output projection ----
            with (
                tc.tile_pool(name="psPV", bufs=1, space="PSUM") as psPV,
                tc.tile_pool(name="psT", bufs=3, space="PSUM") as psT,
                tc.tile_pool(name="psO", bufs=2, space="PSUM") as psO,
                tc.tile_pool(name="psS2", bufs=1, space="PSUM") as psS2,
            ):
                def outproj(tt2, attnT2):
                    ts2 = slice(tt2 * TQ, (tt2 + 1) * TQ)
                    for oc in range(HK):
                        ps = psO.tile([128, TQ], F32, tag="outp")
                        for hd in range(4):
                            nc.tensor.matmul(
                                ps[:], wo_sb[:, hd, oc * 128:(oc + 1) * 128],
                                attnT2[:, hd, :],
                                start=(hd == 0), stop=(hd == 3))
                        ost = ostp.tile([128, TQ], BF16, tag="ost")
                        nc.scalar.copy(ost[:], ps[:])
                        nc.sync.dma_start(
                            outT[oc * 128:(oc + 1) * 128, ts2], ost[:])

                prev = None
                for tt in range(NTQ):
                    ts = slice(tt * TQ, (tt + 1) * TQ)
                    blocks = plan[tt]
                    attnT = attnp.tile([128, 4, TQ], BF16, tag="attnT")
                    for h in range(2):
                        acc0 = psPV.tile([128, TQ], F32, tag="pv0")
                        acc1 = psPV.tile([128, TQ], F32, tag="pv1")
                        den = psS2.tile([1, TQ], F32, tag="den")
                        for bi, (k, lo, hi, mid, mlo, mhi) in enumerate(blocks):
                            w = hi - lo
                            qsl = slice(tt * TQ + lo, tt * TQ + hi)
                            st_ps = psT.tile([128, TQ], F32, tag="sT")
                            ksl = slice(k * TK, (k + 1) * TK)
                            for dh in range(2):
                                nc.tensor.matmul(
                                    st_ps[:, :w], kTn[:, dh, ksl],
                                    qTn[:, 2 * h + dh, qsl],
                                    start=(dh == 0), stop=(dh == 1))
                            probs = w3.tile([128, TQ], BF16, tag="probs")
                            nc.scalar.activation(probs[:, :w], st_ps[:, :w],
                                                 AFT.Exp,
                                                 scale=rnk_all[:, k, :])
                            if mid >= 0:
                                mw = mhi - mlo
                                nc.vector.tensor_mul(
                                    probs[:, mlo - lo:mhi - lo],
                                    probs[:, mlo - lo:mhi - lo],
                                    mask_sb[:, mid, :mw])
                            first, last = (bi == 0), (bi == len(blocks) - 1)
                            with deferred(40):
                                nc.tensor.matmul(den[:, lo:hi], ones_col_bf[:],
                                                 probs[:, :w],
                                                 start=first, stop=last)
                            nc.tensor.matmul(acc0[:, lo:hi], v_sb[:, k, 0:128],
                                             probs[:, :w],
                                             start=first, stop=last)
                            nc.tensor.matmul(acc1[:, lo:hi], v_sb[:, k, 128:256],
                                             probs[:, :w],
                                             start=first, stop=last)
                        a0 = w2.tile([128, TQ], BF16, tag="a0")
                        nc.scalar.copy(a0[:], acc0[:])
                        a1 = w2.tile([128, TQ], BF16, tag="a1")
                        nc.vector.tensor_copy(a1[:], acc1[:])
                        rdf = w1.tile([1, TQ], F32, tag="rdf")
                        nc.vector.reciprocal_approx_fast(rdf[:], den[:])
                        rden = w1.tile([1, TQ], BF16, tag="rden")
                        nc.vector.tensor_copy(rden[:], rdf[:])
                        bc = w2.tile([128, TQ], BF16, tag="bcs2")
                        nc.gpsimd.partition_broadcast(bc[:], rden[:])
                        nc.vector.tensor_mul(attnT[:, 2 * h, :], a0[:], bc[:])
                        nc.vector.tensor_mul(attnT[:, 2 * h + 1, :], a1[:], bc[:])
                    # output projection shifted one tile (overlaps the
                    # next tile's attention with this tile's tail chain)
                    if prev is not None:
                        outproj(tt - 1, prev)
                    prev = attnT
                outproj(NTQ - 1, prev)
    nc.compile()
    return nc


def _maybe_patch_ldw_opt():
    if os.environ.get("LDWOPT", "0") != "1":
        return
    import types
    import subprocess as _sp
    import concourse.bass_utils as bu

    if getattr(bu, "_ldw_patched", False):
        return
    proxy = types.ModuleType("subprocess_ldw")
    proxy.__dict__.update(_sp.__dict__)

    def check_call(cmd, *a, **k):
        if isinstance(cmd, list):
            cmd = [c.replace("--enable-ldw-opt=false", "--enable-ldw-opt=true")
                   if isinstance(c, str) else c for c in cmd]
        return _sp.check_call(cmd, *a, **k)

    proxy.check_call = check_call
    bu.subprocess = proxy
    bu._ldw_patched = True


_CACHE = {}


def _get_nc(key, nmask, plan, apply_qs, apply_ks):
    if key not in _CACHE:
        _CACHE[key] = build_nc(nmask, plan, apply_qs, apply_ks)
    return _CACHE[key]


def _ensure_ntff_hook():
    """Provide the antenv.axon_hooks registry that concourse's axon
    trace path expects; wire it to the ctypes NTFF profiler."""
    import types

    if "antenv.axon_hooks" in sys.modules:
        return
    import antenv

    mod = types.ModuleType("antenv.axon_hooks")
    _h = [None]
    mod.set_axon_ntff_profile_hook = lambda h: _h.__setitem__(0, h)
    mod.get_axon_ntff_profile_hook = lambda: _h[0]
    sys.modules["antenv.axon_hooks"] = mod
    antenv.axon_hooks = mod
    try:
        from trn_agent_boot.trn_boot import _ntff_profile_via_ctypes

        hook = _ntff_profile_via_ctypes("/opt/axon/libaxon_pjrt.so")
        if hook is not None:
            mod.set_axon_ntff_profile_hook(hook)
    except Exception:
        pass
    import concourse.bass_utils as bu

    bu.upload_artifacts = lambda d: f"local://{d}"


def kernel(x, Wq, Wk, Wv, Wo, q_scale, k_scale, segment_ids, mask, cur_ind,
           _trace=False):
    _maybe_patch_ldw_opt()
    x = np.asarray(x, np.float32)
    Wq = np.asarray(Wq, np.float32)
    Wk = np.asarray(Wk, np.float32)
    Wv = np.asarray(Wv, np.float32)
    Wo = np.asarray(Wo, np.float32)
    q_scale = np.asarray(q_scale, np.float32)
    k_scale = np.asarray(k_scale, np.float32)
    seg = np.asarray(segment_ids)
    maskb = np.asarray(mask)

    # positions -> rope tables (host, f64)
    ar = np.arange(T)
    fraction = np.arange(0, D, 2, dtype=np.float32) / D
    freq = (1.0 / (ROPE_THETA ** fraction)).astype(np.float64)
    cos_b = np.empty((B, D // 2, T), np.float32)
    sin_b = np.empty((B, D // 2, T), np.float32)
    for b in range(B):
        row = seg[b]
        p = np.where(row != 0, ar - int(np.argmax(row)), 2 ** 30)
        p = (p + int(np.asarray(cur_ind))).astype(np.float64)
        ang = freq[:, None] * p[None, :]
        cos_b[b] = np.cos(ang)
        sin_b[b] = np.sin(ang)

    m2d = maskb[0, 0]
    same_mask = all(np.array_equal(maskb[b, 0], m2d) for b in range(1, B))
    if not same_mask:
        raise NotImplementedError("per-batch masks not supported")
    patterns, plan = _attn_plan(np.asarray(m2d, bool))
    nmask = patterns.shape[0]
    apply_qs = bool(np.any(q_scale != 0))
    apply_ks = bool(np.any(k_scale != 0))

    plan_key = (nmask, apply_qs, apply_ks,
                tuple(tuple(b) for bl in plan for b in bl))
    nc = _get_nc(plan_key, nmask, plan, apply_qs, apply_ks)

    mask_bf = bf16(patterns.transpose(1, 0, 2))      # [TK, nmask, 128]
    # xT packed to SBUF layout [NTQ, 128(p), HK, TQ]
    xT = [bf16(x[b].T.reshape(HK, 128, NTQ, TQ).transpose(2, 1, 0, 3))
          for b in range(B)]
    pack_w = lambda W: bf16(W.reshape(HK, 128, -1).transpose(1, 0, 2))
    qsc = (1.0 + q_scale).reshape(D, 1).astype(np.float32)
    ksc = (1.0 + k_scale).reshape(D, 1).astype(np.float32)

    in_maps = []
    for c in range(NCORES):
        b, j = divmod(c, NKV)
        in_maps.append({
            "xT": xT[b],
            "wq": pack_w(Wq[:, DQ * j:DQ * (j + 1)]),
            "wk": pack_w(Wk[:, DK * j:DK * (j + 1)]),
            "wv": pack_w(Wv[:, DK * j:DK * (j + 1)]),
            "wo": bf16(Wo[DQ * j:DQ * (j + 1), :]
                       .reshape(4, 128, H).transpose(1, 0, 2)),
            "cosd": bf16(cos_b[b]), "sind": bf16(sin_b[b]),
            "maskt": mask_bf, "qsc": qsc, "ksc": ksc,
        })

    tmpdir = None
    if _trace:
        _ensure_ntff_hook()
        import tempfile

        os.makedirs("/tmp/ntff", exist_ok=True)
        tmpdir = tempfile.mkdtemp(prefix="run", dir="/tmp/ntff")
    res = run_bass_kernel_spmd(nc, in_maps, list(range(NCORES)),
                               trace=_trace, tmpdir=tmpdir)
    if _trace:
        kernel.last_exec_time_ns = res.exec_time_ns
        kernel.last_profile_dir = tmpdir
    outs = [r["outT"].astype(np.float32) for r in res.results]
    out = np.empty((B, T, H), np.float32)
    for b in range(B):
        acc = outs[4 * b]
        for j in range(1, NKV):
            acc = acc + outs[4 * b + j]
        out[b] = acc.T
    return out


# revision 30
# speedup vs baseline: 1.0322x; 1.0322x over previous
"""Gemma3 sliding-window attention on 8 trn2 NeuronCores (Bass/Tile).

Sharding: core c -> batch b=c//4, kv-head j=c%4 (query heads 2j,2j+1).
Each core computes d-major qT/kT projections (no transposes needed for
scores), token-major v, flash-style sT=[tk,tq] attention blocks over the
1024-token sliding window, and a partial output projection outT[2560,2048].
Host sums the 4 partials per batch and transposes back.
"""

import os
import sys

for _p in ("/opt/trn_rl_repo", "/root/.axon_site/_ro/trn_rl_repo"):
    if _p not in sys.path:
        sys.path.append(_p)

import numpy as np
import ml_dtypes

import concourse.bass as bass
import concourse.bacc as bacc
import concourse.mybir as mybir
from concourse.tile import TileContext
from concourse.bass_utils import run_bass_kernel_spmd

B, T, H = 2, 2048, 2560
NH, NKV, D = 8, 4, 256
WINDOW = 1024
EPS = 1e-6
ROPE_THETA = 10000.0
SCALE = D ** -0.5

F32 = mybir.dt.float32
BF16 = mybir.dt.bfloat16

NCORES = 8
HK = 20          # H / 128 contraction tiles
TQ = 512         # query tile (free dim)
NTQ = T // TQ    # 4
TK = 128         # key tile (partition dim)
DQ = 512         # q cols per core (2 heads)
DK = 256         # k/v cols per core (1 kv head)

AFT = mybir.ActivationFunctionType

bf16 = lambda a: np.ascontiguousarray(a).astype(ml_dtypes.bfloat16)


def _attn_plan(mask2d):
    """Static block plan from the [T,T] bool mask.

    Returns (patterns [n,TK,128] float32 0/1 (zero-padded), plan[tt] = list of
    (k, lo, hi, mid, mlo, mhi)): block covers valid tq columns [lo,hi) of the
    tile; mask pattern mid multiplies local columns [mlo,mhi) (width <= 128).
    Blocks in sT layout [tk, tq]."""
    pat_index = {}
    patterns = []
    plan = []
    for tt in range(NTQ):
        q0 = tt * TQ
        blocks = []
        for k in range(T // TK):
            k0 = k * TK
            blk = mask2d[q0:q0 + TQ, k0:k0 + TK]   # [TQ, TK]
            cols_any = blk.any(axis=1)
            if not cols_any.any():
                continue
            idx = np.nonzero(cols_any)[0]
            lo, hi = int(idx[0]), int(idx[-1]) + 1
            assert cols_any[lo:hi].all(), "non-contiguous valid column range"
            cols_all = blk.all(axis=1)
            partial = cols_any & ~cols_all
            if partial.any():
                pidx = np.nonzero(partial)[0]
                mlo, mhi = int(pidx[0]), int(pidx[-1]) + 1
                assert mhi - mlo <= 128, "partial range wider than 128"
                sub = blk.T[:, mlo:mhi].astype(np.float32)   # [TK, w]
                padded = np.zeros((TK, 128), np.float32)
                padded[:, :sub.shape[1]] = sub
                key = padded.tobytes()
                if key not in pat_index:
                    pat_index[key] = len(patterns)
                    patterns.append(padded)
                blocks.append((k, lo, hi, pat_index[key], mlo, mhi))
            else:
                blocks.append((k, lo, hi, -1, 0, 0))
        plan.append(blocks)
    if not patterns:
        patterns = [np.ones((TK, 128), np.float32)]
    return np.stack(patterns), plan


def build_nc(nmask, plan, apply_qs, apply_ks):
    nc = bacc.Bacc("TRN2", target_bir_lowering=False, debug=False)

    xT = nc.dram_tensor("xT", [NTQ, 128, HK, TQ], BF16, kind="ExternalInput")
    wq = nc.dram_tensor("wq", [128, HK, DQ], BF16, kind="ExternalInput")
    wk = nc.dram_tensor("wk", [128, HK, DK], BF16, kind="ExternalInput")
    wv = nc.dram_tensor("wv", [128, HK, DK], BF16, kind="ExternalInput")
    wo = nc.dram_tensor("wo", [128, 4, H], BF16, kind="ExternalInput")
    cosd = nc.dram_tensor("cosd", [D // 2, T], BF16, kind="ExternalInput")
    sind = nc.dram_tensor("sind", [D // 2, T], BF16, kind="ExternalInput")
    maskt = nc.dram_tensor("maskt", [128, nmask, 128], BF16, kind="ExternalInput")
    qsc = nc.dram_tensor("qsc", [D, 1], F32, kind="ExternalInput")
    ksc = nc.dram_tensor("ksc", [D, 1], F32, kind="ExternalInput")
    outT = nc.dram_tensor("outT", [H, T], BF16, kind="ExternalOutput")

    with TileContext(nc) as tc:
        nc_lp = nc.allow_low_precision(reason="bf16 throughout; tol 2e-2")
        nc_lp.__enter__()
        with (
            tc.tile_pool(name="const", bufs=1) as constp,
            tc.tile_pool(name="wts", bufs=1) as wtsp,
            tc.tile_pool(name="xt", bufs=2) as xtp,
            tc.tile_pool(name="qkv", bufs=1) as qkvp,
            tc.tile_pool(name="w1", bufs=1) as w1,
            tc.tile_pool(name="w2", bufs=2) as w2,
            tc.tile_pool(name="w3", bufs=3) as w3,
            tc.tile_pool(name="ostp", bufs=6) as ostp,
            tc.tile_pool(name="attn", bufs=2) as attnp,
        ):
            from contextlib import contextmanager

            @contextmanager
            def deferred(off=200000):
                tc.cur_priority += off
                try:
                    yield
                finally:
                    tc.cur_priority -= off

            # ---- static loads (ordered by first use) ----
            wq_sb = wtsp.tile([128, HK, DQ], BF16, tag="wq")
            for c0, c1 in ((0, 2), (2, 7), (7, 13), (13, 20)):
                nc.sync.dma_start(wq_sb[:, c0:c1, :], wq[:, c0:c1, :])
            wk_sb = wtsp.tile([128, HK, DK], BF16, tag="wk")
            wv_sb = wtsp.tile([128, HK, DK], BF16, tag="wv")
            cos_sb = constp.tile([128, T], BF16, tag="cos")
            sin_sb = constp.tile([128, T], BF16, tag="sin")
            qsc_sb = constp.tile([128, 2, 1], F32, tag="qsc")
            ksc_sb = constp.tile([128, 2, 1], F32, tag="ksc")
            with deferred(60):
                nc.sync.dma_start(wk_sb[:], wk[:])
                nc.sync.dma_start(wv_sb[:], wv[:])
                nc.sync.dma_start(cos_sb[:], cosd[:])
                nc.sync.dma_start(sin_sb[:], sind[:])
                nc.sync.dma_start(qsc_sb[:],
                                  qsc.rearrange("(n p) o -> p n o", p=128))
                nc.sync.dma_start(ksc_sb[:],
                                  ksc.rearrange("(n p) o -> p n o", p=128))
            mask_sb = constp.tile([128, nmask, 128], BF16, tag="mask")
            wo_sb = wtsp.tile([128, 4, H], BF16, tag="wo")
            with deferred(2000):
                nc.sync.dma_start(mask_sb[:], maskt[:])
                nc.sync.dma_start(wo_sb[:], wo[:])

            ones_col = constp.tile([128, 1], F32, tag="ones_col")
            nc.vector.memset(ones_col[:], 1.0)
            ones_col_bf = constp.tile([128, 1], BF16, tag="ones_col_bf")
            nc.vector.memset(ones_col_bf[:], 1.0)
            ones_row = constp.tile([1, 128], F32, tag="ones_row")
            nc.vector.memset(ones_row[:], 1.0)
            eps_sb = constp.tile([1, 1], F32, tag="eps")
            nc.vector.memset(eps_sb[:], EPS)
            eps_sb128 = constp.tile([128, 1], F32, tag="eps128")
            nc.vector.memset(eps_sb128[:], EPS)
            epsD_sb = constp.tile([128, 1], F32, tag="epsD")
            nc.vector.memset(epsD_sb[:], float(D) * EPS)

            # PE warmup while startup DMAs stream (HAM un-throttle)
            with tc.tile_pool(name="warm", bufs=1) as warmp, \
                 tc.tile_pool(name="warmps", bufs=1, space="PSUM") as warmpsp:
                wsrc = warmp.tile([128, 512], BF16, tag="wsrc")
                nc.vector.memset(wsrc[:], 0.0)
                wps = warmpsp.tile([128, 512], F32, tag="wps")
                for wi in range(20):
                    nc.tensor.matmul(wps[:], wsrc[:, 0:128], wsrc[:],
                                     start=(wi == 0), stop=(wi == 19))

            # persistent per-core tensors
            qTn = qkvp.tile([128, 4, T], BF16, tag="qTn")    # d-major, 4 d-tiles
            kTn = qkvp.tile([128, 2, T], BF16, tag="kTn")    # d-major, 2 d-tiles
            v_sb = qkvp.tile([128, T // TK, DK], BF16, tag="v")  # token-major

            # ---- phase 1: projections + rmsnorm + rope ----
            def rope_q(ps_d0, ps_d1, cos_t, sin_t, out, o0, o1, sc, apply_sc):
                """q path: evacuate PSUM, rmsnorm (PE colsum + gpsimd bcast),
                rope, normalize, write bf16 d-major."""
                q0 = w2.tile([128, TQ], BF16, tag="qp0")
                nc.scalar.copy(q0[:], ps_d0[:])
                q1 = w2.tile([128, TQ], BF16, tag="qp1")
                nc.vector.tensor_copy(q1[:], ps_d1[:])
                sq0 = w2.tile([128, TQ], BF16, tag="sq0")
                nc.vector.tensor_mul(sq0[:], q0[:], q0[:])
                sq1 = w2.tile([128, TQ], BF16, tag="sq1")
                nc.vector.tensor_mul(sq1[:], q1[:], q1[:])
                ssq = psS.tile([1, TQ], F32, tag="ssq")
                with deferred(500):
                    nc.tensor.matmul(ssq[:], ones_col_bf[:], sq0[:],
                                     start=True, stop=False)
                    nc.tensor.matmul(ssq[:], ones_col_bf[:], sq1[:],
                                     start=False, stop=True)
                rnorm = w1.tile([1, TQ], BF16, tag="rnorm")
                nc.scalar.activation(rnorm[:], ssq[:], AFT.Abs_reciprocal_sqrt,
                                     bias=eps_sb[:], scale=1.0 / D)
                bc = w2.tile([128, TQ], BF16, tag="bcs")
                nc.gpsimd.partition_broadcast(bc[:], rnorm[:])
                # rope halves: h0' = h0*cos - h1*sin ; h1' = h1*cos + h0*sin
                t0 = w2.tile([128, TQ], BF16, tag="t0")
                nc.vector.tensor_mul(t0[:], q0[:], cos_t)
                t1 = w2.tile([128, TQ], BF16, tag="t1")
                nc.vector.tensor_mul(t1[:], q1[:], sin_t)
                nc.vector.tensor_sub(t0[:], t0[:], t1[:])
                if apply_sc:
                    nc.vector.tensor_scalar_mul(t0[:], t0[:], sc[:, 0, :])
                nc.vector.tensor_mul(out[:, o0, :], t0[:], bc[:])
                t2 = w2.tile([128, TQ], BF16, tag="t2")
                nc.vector.tensor_mul(t2[:], q1[:], cos_t)
                t3 = w2.tile([128, TQ], BF16, tag="t3")
                nc.vector.tensor_mul(t3[:], q0[:], sin_t)
                nc.vector.tensor_add(t2[:], t2[:], t3[:])
                if apply_sc:
                    nc.vector.tensor_scalar_mul(t2[:], t2[:], sc[:, 1, :])
                nc.vector.tensor_mul(out[:, o1, :], t2[:], bc[:])

            def rope_k(ps_d0, ps_d1, cos_t, sin_t, out, sc, apply_sc, tt):
                """k path: rope only; k-norm is folded into the exp scale
                later. Computes per-key 1/rms columns into rnk_all."""
                q0 = w2.tile([128, TQ], BF16, tag="qp0")
                nc.scalar.copy(q0[:], ps_d0[:])
                q1 = w2.tile([128, TQ], BF16, tag="qp1")
                nc.vector.tensor_copy(q1[:], ps_d1[:])
                t0 = w2.tile([128, TQ], BF16, tag="t0")
                nc.vector.tensor_mul(t0[:], q0[:], cos_t)
                t1 = w2.tile([128, TQ], BF16, tag="t1")
                nc.vector.tensor_mul(t1[:], q1[:], sin_t)
                if apply_sc:
                    nc.vector.tensor_sub(t0[:], t0[:], t1[:])
                    nc.vector.tensor_scalar_mul(out[:, 0, :], t0[:], sc[:, 0, :])
                else:
                    nc.vector.tensor_sub(out[:, 0, :], t0[:], t1[:])
                t2 = w2.tile([128, TQ], BF16, tag="t2")
                nc.vector.tensor_mul(t2[:], q1[:], cos_t)
                t3 = w2.tile([128, TQ], BF16, tag="t3")
                nc.vector.tensor_mul(t3[:], q0[:], sin_t)
                if apply_sc:
                    nc.vector.tensor_add(t2[:], t2[:], t3[:])
                    nc.vector.tensor_scalar_mul(out[:, 1, :], t2[:], sc[:, 1, :])
                else:
                    nc.vector.tensor_add(out[:, 1, :], t2[:], t3[:])
                # per-key 1/rms columns (post-rope k; rope preserves norms
                # exactly, and scale multiplies elementwise after norm)
                sk0 = w2.tile([128, TQ], BF16, tag="sk0")
                nc.vector.tensor_mul(sk0[:], out[:, 0, :], out[:, 0, :])
                sk1 = w2.tile([128, TQ], BF16, tag="sk1")
                nc.vector.tensor_mul(sk1[:], out[:, 1, :], out[:, 1, :])
                for st in range(TQ // TK):
                    ssqk = psK.tile([128, 1], F32, tag="ssqk")
                    sl = slice(st * TK, (st + 1) * TK)
                    with deferred(300):
                        nc.tensor.matmul(ssqk[:], sk0[:, sl], ones_col_bf[:],
                                         start=True, stop=False)
                        nc.tensor.matmul(ssqk[:], sk1[:, sl], ones_col_bf[:],
                                         start=False, stop=True)
                    nc.scalar.activation(rnk_all[:, tt * 4 + st], ssqk[:],
                                         AFT.Abs_reciprocal_sqrt,
                                         bias=epsD_sb[:], scale=1.0)

            rnk_all = constp.tile([128, T // TK, 1], F32, tag="rnk")
            with (
                tc.tile_pool(name="psP", bufs=3, space="PSUM") as psP,
                tc.tile_pool(name="psV", bufs=2, space="PSUM") as psV,
                tc.tile_pool(name="psS1", bufs=1, space="PSUM") as psS,
                tc.tile_pool(name="psK", bufs=2, space="PSUM") as psK,
            ):
                for tt in range(NTQ):
                    ts = slice(tt * TQ, (tt + 1) * TQ)
                    xt_t = xtp.tile([128, HK, TQ], BF16, tag="xt")
                    xt_chunks = ((0, 3), (3, 8), (8, 14), (14, 20)) if tt == 0 \
                        else ((0, 10), (10, 20))
                    for c0, c1 in xt_chunks:
                        nc.sync.dma_start(xt_t[:, c0:c1, :], xT[tt, :, c0:c1, :])
                    cos_t = cos_sb[:, ts]
                    sin_t = sin_sb[:, ts]
                    # q heads: d-tiles (2h, 2h+1) per head h
                    for h in range(2):
                        pd = []
                        for dh in range(2):
                            dt = 2 * h + dh
                            ps = psP.tile([128, TQ], F32, tag="projq")
                            for ki in range(HK):
                                nc.tensor.matmul(
                                    ps[:],
                                    wq_sb[:, ki, dt * 128:(dt + 1) * 128],
                                    xt_t[:, ki, :],
                                    start=(ki == 0), stop=(ki == HK - 1))
                            pd.append(ps)
                        rope_q(pd[0], pd[1], cos_t, sin_t,
                               qTn[:, :, ts], 2 * h, 2 * h + 1,
                               qsc_sb, apply_qs)
                    # k
                    pd = []
                    for dh in range(2):
                        ps = psP.tile([128, TQ], F32, tag="projq")
                        for ki in range(HK):
                            nc.tensor.matmul(
                                ps[:], wk_sb[:, ki, dh * 128:(dh + 1) * 128],
                                xt_t[:, ki, :],
                                start=(ki == 0), stop=(ki == HK - 1))
                        pd.append(ps)
                    rope_k(pd[0], pd[1], cos_t, sin_t,
                           kTn[:, :, ts], ksc_sb, apply_ks, tt)
                    # v (token-major)
                    for st in range(TQ // TK):
                        ps = psV.tile([128, DK], F32, tag="projv")
                        for ki in range(HK):
                            nc.tensor.matmul(
                                ps[:], xt_t[:, ki, st * 128:(st + 1) * 128],
                                wv_sb[:, ki, :],
                                start=(ki == 0), stop=(ki == HK - 1))
                        nc.vector.tensor_copy(v_sb[:, tt * 4 + st, :], ps[:])

            # ---- phase 2: attention + output projection ----
            with (
                tc.tile_pool(name="psPV", bufs=1, space="PSUM") as psPV,
                tc.tile_pool(name="psT", bufs=3, space="PSUM") as psT,
                tc.tile_pool(name="psO", bufs=2, space="PSUM") as psO,
                tc.tile_pool(name="psS2", bufs=1, space="PSUM") as psS2,
            ):
                def outproj(tt2, attnT2):
                    ts2 = slice(tt2 * TQ, (tt2 + 1) * TQ)
                    for oc in range(HK):
                        ps = psO.tile([128, TQ], F32, tag="outp")
                        for hd in range(4):
                            nc.tensor.matmul(
                                ps[:], wo_sb[:, hd, oc * 128:(oc + 1) * 128],
                                attnT2[:, hd, :],
                                start=(hd == 0), stop=(hd == 3))
                        ost = ostp.tile([128, TQ], BF16, tag="ost")
                        nc.scalar.copy(ost[:], ps[:])
                        nc.sync.dma_start(
                            outT[oc * 128:(oc + 1) * 128, ts2], ost[:])

                prev = None
                for tt in range(NTQ):
                    ts = slice(tt * TQ, (tt + 1) * TQ)
                    blocks = plan[tt]
                    attnT = attnp.tile([128, 4, TQ], BF16, tag="attnT")
                    for h in range(2):
                        acc0 = psPV.tile([128, TQ], F32, tag="pv0")
                        acc1 = psPV.tile([128, TQ], F32, tag="pv1")
                        den = psS2.tile([1, TQ], F32, tag="den")
                        for bi, (k, lo, hi, mid, mlo, mhi) in enumerate(blocks):
                            w = hi - lo
                            qsl = slice(tt * TQ + lo, tt * TQ + hi)
                            st_ps = psT.tile([128, TQ], F32, tag="sT")
                            ksl = slice(k * TK, (k + 1) * TK)
                            for dh in range(2):
                                nc.tensor.matmul(
                                    st_ps[:, :w], kTn[:, dh, ksl],
                                    qTn[:, 2 * h + dh, qsl],
                                    start=(dh == 0), stop=(dh == 1))
                            probs = w3.tile([128, TQ], BF16, tag="probs")
                            nc.scalar.activation(probs[:, :w], st_ps[:, :w],
                                                 AFT.Exp,
                                                 scale=rnk_all[:, k, :])
                            if mid >= 0:
                                mw = mhi - mlo
                                nc.vector.tensor_mul(
                                    probs[:, mlo - lo:mhi - lo],
                                    probs[:, mlo - lo:mhi - lo],
                                    mask_sb[:, mid, :mw])
                            first, last = (bi == 0), (bi == len(blocks) - 1)
                            with deferred(40):
                                nc.tensor.matmul(den[:, lo:hi], ones_col_bf[:],
                                                 probs[:, :w],
                                                 start=first, stop=last)
                            nc.tensor.matmul(acc0[:, lo:hi], v_sb[:, k, 0:128],
                                             probs[:, :w],
                                             start=first, stop=last)
                            nc.tensor.matmul(acc1[:, lo:hi], v_sb[:, k, 128:256],
                                             probs[:, :w],
                                             start=first, stop=last)
                        a0 = w2.tile([128, TQ], BF16, tag="a0")
                        nc.scalar.copy(a0[:], acc0[:])
                        a1 = w2.tile([128, TQ], BF16, tag="a1")
                        nc.vector.tensor_copy(a1[:], acc1[:])
                        rdf = w1.tile([1, TQ], F32, tag="rdf")
                        nc.vector.reciprocal_approx_fast(rdf[:], den[:])
                        rden = w1.tile([1, TQ], BF16, tag="rden")
                        nc.vector.tensor_copy(rden[:], rdf[:])
                        bc = w2.tile([128, TQ], BF16, tag="bcs2")
                        nc.gpsimd.partition_broadcast(bc[:], rden[:])
                        nc.vector.tensor_mul(attnT[:, 2 * h, :], a0[:], bc[:])
                        nc.vector.tensor_mul(attnT[:, 2 * h + 1, :], a1[:], bc[:])
                    # output projection shifted one tile (overlaps the
                    # next tile's attention with this tile's tail chain)
                    if prev is not None:
                        outproj(tt - 1, prev)
                    prev = attnT
                outproj(NTQ - 1, prev)
    nc.compile()
    return nc


def _maybe_patch_ldw_opt():
    if os.environ.get("LDWOPT", "0") != "1":
        return
    import types
    import subprocess as _sp
    import concourse.bass_utils as bu

    if getattr(bu, "_ldw_patched", False):
        return
    proxy = types.ModuleType("subprocess_ldw")
    proxy.__dict__.update(_sp.__dict__)

    def check_call(cmd, *a, **k):
        if isinstance(cmd, list):
            cmd = [c.replace("--enable-ldw-opt=false", "--enable-ldw-opt=true")
                   if isinstance(c, str) else c for c in cmd]
        return _sp.check_call(cmd, *a, **k)

    proxy.check_call = check_call
    bu.subprocess = proxy
    bu._ldw_patched = True


_CACHE = {}


def _get_nc(key, nmask, plan, apply_qs, apply_ks):
    if key not in _CACHE:
        _CACHE[key] = build_nc(nmask, plan, apply_qs, apply_ks)
    return _CACHE[key]


def _ensure_ntff_hook():
    """Provide the antenv.axon_hooks registry that concourse's axon
    trace path expects; wire it to the ctypes NTFF profiler."""
    import types

    if "antenv.axon_hooks" in sys.modules:
        return
    import antenv

    mod = types.ModuleType("antenv.axon_hooks")
    _h = [None]
    mod.set_axon_ntff_profile_hook = lambda h: _h.__setitem__(0, h)
    mod.get_axon_ntff_profile_hook = lambda: _h[0]
    sys.modules["antenv.axon_hooks"] = mod
    antenv.axon_hooks = mod
    try:
        from trn_agent_boot.trn_boot import _ntff_profile_via_ctypes

        hook = _ntff_profile_via_ctypes("/opt/axon/libaxon_pjrt.so")
        if hook is not None:
            mod.set_axon_ntff_profile_hook(hook)
    except Exception:
        pass
    import concourse.bass_utils as bu

    bu.upload_artifacts = lambda d: f"local://{d}"


def kernel(x, Wq, Wk, Wv, Wo, q_scale, k_scale, segment_ids, mask, cur_ind,
           _trace=False):
    _maybe_patch_ldw_opt()
    x = np.asarray(x, np.float32)
    Wq = np.asarray(Wq, np.float32)
    Wk = np.asarray(Wk, np.float32)
    Wv = np.asarray(Wv, np.float32)
    Wo = np.asarray(Wo, np.float32)
    q_scale = np.asarray(q_scale, np.float32)
    k_scale = np.asarray(k_scale, np.float32)
    seg = np.asarray(segment_ids)
    maskb = np.asarray(mask)

    # positions -> rope tables (host, f64)
    ar = np.arange(T)
    fraction = np.arange(0, D, 2, dtype=np.float32) / D
    freq = (1.0 / (ROPE_THETA ** fraction)).astype(np.float64)
    cos_b = np.empty((B, D // 2, T), np.float32)
    sin_b = np.empty((B, D // 2, T), np.float32)
    for b in range(B):
        row = seg[b]
        p = np.where(row != 0, ar - int(np.argmax(row)), 2 ** 30)
        p = (p + int(np.asarray(cur_ind))).astype(np.float64)
        ang = freq[:, None] * p[None, :]
        cos_b[b] = np.cos(ang)
        sin_b[b] = np.sin(ang)

    m2d = maskb[0, 0]
    same_mask = all(np.array_equal(maskb[b, 0], m2d) for b in range(1, B))
    if not same_mask:
        raise NotImplementedError("per-batch masks not supported")
    patterns, plan = _attn_plan(np.asarray(m2d, bool))
    nmask = patterns.shape[0]
    apply_qs = bool(np.any(q_scale != 0))
    apply_ks = bool(np.any(k_scale != 0))

    plan_key = (nmask, apply_qs, apply_ks,
                tuple(tuple(b) for bl in plan for b in bl))
    nc = _get_nc(plan_key, nmask, plan, apply_qs, apply_ks)

    mask_bf = bf16(patterns.transpose(1, 0, 2))      # [TK, nmask, 128]
    # xT packed to SBUF layout [NTQ, 128(p), HK, TQ]
    xT = [bf16(x[b].T.reshape(HK, 128, NTQ, TQ).transpose(2, 1, 0, 3))
          for b in range(B)]
    pack_w = lambda W: bf16(W.reshape(HK, 128, -1).transpose(1, 0, 2))
    qsc = (1.0 + q_scale).reshape(D, 1).astype(np.float32)
    ksc = (1.0 + k_scale).reshape(D, 1).astype(np.float32)

    in_maps = []
    for c in range(NCORES):
        b, j = divmod(c, NKV)
        in_maps.append({
            "xT": xT[b],
            "wq": pack_w(Wq[:, DQ * j:DQ * (j + 1)]),
            "wk": pack_w(Wk[:, DK * j:DK * (j + 1)]),
            "wv": pack_w(Wv[:, DK * j:DK * (j + 1)]),
            "wo": bf16(Wo[DQ * j:DQ * (j + 1), :]
                       .reshape(4, 128, H).transpose(1, 0, 2)),
            "cosd": bf16(cos_b[b]), "sind": bf16(sin_b[b]),
            "maskt": mask_bf, "qsc": qsc, "ksc": ksc,
        })

    tmpdir = None
    if _trace:
        _ensure_ntff_hook()
        import tempfile

        os.makedirs("/tmp/ntff", exist_ok=True)
        tmpdir = tempfile.mkdtemp(prefix="run", dir="/tmp/ntff")
    res = run_bass_kernel_spmd(nc, in_maps, list(range(NCORES)),
                               trace=_trace, tmpdir=tmpdir)
    if _trace:
        kernel.last_exec_time_ns = res.exec_time_ns
        kernel.last_profile_dir = tmpdir
    outs = [r["outT"].astype(np.float32) for r in res.results]
    out = np.empty((B, T, H), np.float32)
    for b in range(B):
        acc = outs[4 * b]
        for j in range(1, NKV):
            acc = acc + outs[4 * b + j]
        out[b] = acc.T
    return out
